# revision 30
# baseline (speedup 1.0000x reference)
"""Causal multi-head attention (B=8, T=1024, E=768, H=12, D=64) on 8 trn2
NeuronCores, data-parallel over the batch (one batch element per core).

v3: all-bf16 matmuls + software-pipelined emission. The attention inner
loop is ACT(exp)-bound (~11us/pair vs ~7.7us PE), so the next pair's
Q/K projection matmuls are interleaved into the attention emission as PE
filler, and each kc's ctx matmul is emitted 2 steps after its scores so
the exp has drained by the time PE reaches it (the PE queue is in-order).
Softmax denominators ride as a ones column in V ("V65"); both heads'
reciprocal rows are batched into ONE DRAM round-trip per window (sc
issued from the sync queue, bc + normalize mults on Pool) so only a
single DMA-completion wait sits on the Pool queue per window.

Per-core pipeline:
  1. Q^T = Wq @ x^T + bq (bias+cast ACT), K^T likewise (DVE) -> bf16
     [768, 1024] (e_out on partitions).
  2. V = x @ Wv^T -> bf16 "V65" [1024, 12*65] with a ones column per head
     so the ctx matmul also produces the softmax denominator (bv folded
     into the output projection bias).
  3. Per head pair: S^T[k,q] row-packed K=64 matmuls, causal mask added on
     the diagonal block (single strided DVE op over both heads), exp on
     ACT (scale=1/8) -> bf16 P^T, ctx^T[65,q] accumulated with V65
     stationary. Row 64 = denominator; reciprocal (DVE) -> DRAM-bounce
     broadcast -> normalize mult (Pool, out bf16).
  4. out = ctx_norm @ Wo^T + bo_eff, bo_eff = bo + bv @ Wo^T.
"""
import sys
import numpy as np

sys.path.insert(0, "/opt/trn_rl_repo")

import concourse.bass as bass
import concourse.mybir as mybir
import concourse.tile as tile

F32 = mybir.dt.float32
BF16 = mybir.dt.bfloat16

B, T, E, H, D = 8, 1024, 768, 12, 64
NCH = E // 128          # 6 e-chunks
NTC = T // 128          # 8 t-chunks
NW = T // 512           # 2 q-windows
SCALE = 1.0 / np.sqrt(D)
NEG = -1.0e9


def _bf16(a):
    import ml_dtypes
    return np.ascontiguousarray(a.astype(ml_dtypes.bfloat16))


def _split_excess_waits(nc, max_waits: int = 1):
    """walrus on this stack accepts at most one embedded sync-wait per
    instruction; peel extras onto wait-only NoOps on the same engine."""
    for func in nc.m.functions:
        for bb in func.blocks:
            insts = bb.instructions
            i = 0
            while i < len(insts):
                inst = insts[i]
                si = getattr(inst, "sync_info", None)
                if si is None or len(si.on_wait) <= max_waits:
                    i += 1
                    continue
                waits = list(si.on_wait)
                keep, extra = waits[:max_waits], waits[max_waits:]
                nops = []
                while extra:
                    chunk, extra = extra[:max_waits], extra[max_waits:]
                    nop = mybir.InstNoOp(
                        name=f"{inst.name}_ws{len(nops)}", ins=[], outs=[])
                    nop.engine = inst.engine
                    nop.sync_info = mybir.SyncInfo(on_wait=chunk, on_update=[])
                    nc.register_instruction(nop, overwrite=True)
                    nops.append(nop)
                si.on_wait = keep
                for j, nop in enumerate(nops):
                    insts.insert(i + j, nop)
                i += len(nops) + 1


def _mix(main_gen, filler_gen, nf=2):
    """Pull 1 step from main, then up to nf from filler, repeat; drain both."""
    main_it, fill_it = iter(main_gen), iter(filler_gen)
    main_done = fill_done = False
    while not (main_done and fill_done):
        if not main_done:
            try:
                next(main_it)
            except StopIteration:
                main_done = True
        if not fill_done:
            for _ in range(nf):
                try:
                    next(fill_it)
                except StopIteration:
                    fill_done = True
                    break


def _chain(*gens):
    for g in gens:
        yield from g


def build_nc(repeat: int = 1):
    import ml_dtypes
    nc = bass.Bass()
    xT = nc.dram_tensor("xT", [E, T], BF16, kind="ExternalInput")
    wq_r = nc.dram_tensor("wq_r", [NCH, 128, NCH, 128], BF16, kind="ExternalInput")
    wk_r = nc.dram_tensor("wk_r", [NCH, 128, NCH, 128], BF16, kind="ExternalInput")
    wvT = nc.dram_tensor("wvT", [E, E], BF16, kind="ExternalInput")
    woT = nc.dram_tensor("woT", [E, E], BF16, kind="ExternalInput")
    bq_pm = nc.dram_tensor("bq_pm", [128, NCH], F32, kind="ExternalInput")
    bk_pm = nc.dram_tensor("bk_pm", [128, NCH], F32, kind="ExternalInput")
    bo_bc = nc.dram_tensor("bo_bc", [128, E], F32, kind="ExternalInput")
    out = nc.dram_tensor("out", [T, E], F32, kind="ExternalOutput")

    tril = np.where(np.arange(128)[None, :] >= np.arange(128)[:, None],
                    0.0, NEG).astype(np.float32)
    maskc = nc.inline_tensor(tril, name="maskc")
    ones12 = nc.inline_tensor(np.ones((128, H), ml_dtypes.bfloat16),
                              name="ones12")

    with tile.TileContext(nc) as tc:
        from contextlib import ExitStack
        with ExitStack() as ctx:
            consts = ctx.enter_context(tc.tile_pool(name="consts", bufs=1))
            persist = ctx.enter_context(tc.tile_pool(name="persist", bufs=1))
            wqk_p = ctx.enter_context(tc.tile_pool(name="wqk", bufs=3))
            wrow_p = ctx.enter_context(tc.tile_pool(name="wrow", bufs=6))
            pt_p = ctx.enter_context(tc.tile_pool(name="pt", bufs=5))
            bc_p = ctx.enter_context(tc.tile_pool(name="bc", bufs=4))
            cu_p = ctx.enter_context(tc.tile_pool(name="cu", bufs=6))
            rt_p = ctx.enter_context(tc.tile_pool(name="rt", bufs=4))
            out_p = ctx.enter_context(tc.tile_pool(name="outp", bufs=3))
            pp = ctx.enter_context(tc.tile_pool(name="pp", bufs=2, space="PSUM"))
            stp = ctx.enter_context(tc.tile_pool(name="stp", bufs=2, space="PSUM"))
            ctxp = ctx.enter_context(tc.tile_pool(name="ctxp", bufs=2, space="PSUM"))
            drp = ctx.enter_context(tc.tile_pool(name="drp", bufs=4, space="DRAM"))

            def body():
                mask2 = consts.tile([128, 2, 128], F32)
                bqs = consts.tile([128, NCH], F32)
                bks = consts.tile([128, NCH], F32)
                bos = consts.tile([128, E], F32)

                xt_sb = persist.tile([128, NCH, T], BF16)
                xq_engines = [nc.sync, nc.gpsimd, nc.scalar,
                              nc.sync, nc.gpsimd, nc.scalar]
                for ch in range(NCH):
                    xq_engines[ch].dma_start(
                        out=xt_sb[:, ch, :],
                        in_=xT[ch * 128:(ch + 1) * 128, :])

                qt_sb = persist.tile([128, NCH, T], BF16)
                kt_sb = persist.tile([128, NCH, T], BF16)
                v65_sb = persist.tile([128, NTC, H * 65], BF16)
                ctxT_sb = persist.tile([128, NCH, T], BF16)

                def proj_qk_steps(m, w_r, bias_sb, dst_sb, copy_eng):
                    w = wqk_p.tile([128, NCH, 128], BF16, tag="wqk")
                    nc.sync.dma_start(out=w, in_=w_r[m, :, :, :])
                    for win in range(NW):
                        ps = pp.tile([128, 512], F32, tag="pp")
                        for ch in range(NCH):
                            nc.tensor.matmul(
                                ps, w[:, ch, :],
                                xt_sb[:, ch, win * 512:(win + 1) * 512],
                                start=(ch == 0), stop=(ch == NCH - 1))
                            yield
                        if copy_eng == "act":
                            nc.scalar.activation(
                                dst_sb[:, m, win * 512:(win + 1) * 512], ps,
                                mybir.ActivationFunctionType.Identity,
                                bias=bias_sb[:, m:m + 1])
                        else:
                            nc.vector.tensor_scalar_add(
                                dst_sb[:, m, win * 512:(win + 1) * 512], ps,
                                bias_sb[:, m:m + 1])

                def proj_v_steps(kcs):
                    for kc in kcs:
                        ps0 = pp.tile([128, 512], F32, tag="pp")
                        ps1 = pp.tile([128, 256], F32, tag="pp")
                        for ch in range(NCH):
                            lhsT = xt_sb[:, ch, kc * 128:(kc + 1) * 128]
                            nc.tensor.matmul(ps0, lhsT, wv_t[ch][:, 0:512],
                                             start=(ch == 0), stop=(ch == NCH - 1))
                            nc.tensor.matmul(ps1, lhsT, wv_t[ch][:, 512:768],
                                             start=(ch == 0), stop=(ch == NCH - 1))
                            yield
                        v65_r = v65_sb[:, kc, :].rearrange("p (h e) -> p h e", e=65)
                        if kc % 2 == 0:
                            nc.scalar.copy(v65_r[:, 0:8, 0:64], ps0)
                            nc.scalar.copy(v65_r[:, 8:12, 0:64], ps1)
                        else:
                            nc.vector.tensor_copy(v65_r[:, 0:8, 0:64], ps0)
                            nc.vector.tensor_copy(v65_r[:, 8:12, 0:64], ps1)

                def attn_pair_steps(p, wins=tuple(range(NW)), delay=3):
                    for win in wins:
                        nk = 4 * (win + 1)
                        ctxA = ctxp.tile([65, 512], F32, tag="ctx")
                        ctxB = ctxp.tile([65, 512], F32, tag="ctx")
                        hA, hB = 2 * p, 2 * p + 1

                        def emit_ctx(kc, off, pt):
                            nc.tensor.matmul(
                                ctxA[:, off:512],
                                v65_sb[:, kc, hA * 65:hA * 65 + 65],
                                pt[:, off:512],
                                start=(kc == 0), stop=(kc == nk - 1))
                            nc.tensor.matmul(
                                ctxB[:, off:512],
                                v65_sb[:, kc, hB * 65:hB * 65 + 65],
                                pt[:, 512 + off:1024],
                                start=(kc == 0), stop=(kc == nk - 1))

                        pend = []
                        for kc in range(nk):
                            off = max(kc * 128 - win * 512, 0)
                            w0 = win * 512
                            st = stp.tile([128, 1024], F32, tag="st")
                            nc.tensor.matmul(
                                st[:, off:512],
                                kt_sb[0:64, p, kc * 128:(kc + 1) * 128],
                                qt_sb[0:64, p, w0 + off:w0 + 512],
                                start=True, stop=True, tile_position=(0, 0))
                            nc.tensor.matmul(
                                st[:, 512 + off:1024],
                                kt_sb[64:128, p, kc * 128:(kc + 1) * 128],
                                qt_sb[64:128, p, w0 + off:w0 + 512],
                                start=True, stop=True, tile_position=(64, 0))
                            if kc * 128 - win * 512 >= 0:
                                st3 = st.rearrange("p (s q) -> p s q", s=2)
                                nc.vector.tensor_tensor(
                                    out=st3[:, :, off:off + 128],
                                    in0=st3[:, :, off:off + 128],
                                    in1=mask2, op=mybir.AluOpType.add)
                            pt = pt_p.tile([128, 1024], BF16, tag="pt")
                            if off > 0:
                                st3 = st.rearrange("p (s q) -> p s q", s=2)
                                pt3 = pt.rearrange("p (s q) -> p s q", s=2)
                                nc.scalar.activation(
                                    pt3[:, :, off:512], st3[:, :, off:512],
                                    mybir.ActivationFunctionType.Exp, scale=SCALE)
                            else:
                                nc.scalar.activation(
                                    pt, st, mybir.ActivationFunctionType.Exp,
                                    scale=SCALE)
                            pend.append((kc, off, pt))
                            if len(pend) > delay:
                                emit_ctx(*pend.pop(0))
                            yield
                        while pend:
                            emit_ctx(*pend.pop(0))
                            yield
                        # one DRAM round-trip per window for BOTH heads:
                        # recips into one [1,1024] row, sc issued from the
                        # sync queue (no wait), bc + mults on Pool (a single
                        # wait-for-sc per window instead of two chains)
                        rt2 = rt_p.tile([1, 1024], F32, tag="rt")
                        cus = []
                        for idx, cps in ((0, ctxA), (1, ctxB)):
                            nc.vector.reciprocal(
                                rt2[:, idx * 512:(idx + 1) * 512],
                                cps[64:65, :])
                            cu = cu_p.tile([65, 512], F32, tag="cu")
                            nc.vector.tensor_copy(cu, cps)
                            cus.append(cu)
                        sc = drp.tile([1, 1024], F32, tag="dr")
                        nc.sync.dma_start(out=sc, in_=rt2)
                        bc = bc_p.tile([64, 1024], F32, tag="bc")
                        sc_b = bass.AP(tensor=sc.tensor, offset=sc.offset,
                                       ap=[[0, 64]] + list(sc.ap)[1:])
                        nc.gpsimd.dma_start(out=bc, in_=sc_b)
                        for idx in (0, 1):
                            nc.gpsimd.tensor_tensor(
                                out=ctxT_sb[idx * 64:idx * 64 + 64, p,
                                            win * 512:(win + 1) * 512],
                                in0=cus[idx][0:64, :],
                                in1=bc[:, idx * 512:(idx + 1) * 512],
                                op=mybir.AluOpType.mult)
                        yield

                def outproj_steps(tcns):
                    for tcn in tcns:
                        ps0 = pp.tile([128, 512], F32, tag="pp")
                        ps1 = ctxp.tile([128, 256], F32, tag="ctx")
                        for ch in range(NCH):
                            lhsT = ctxT_sb[:, ch, tcn * 128:(tcn + 1) * 128]
                            nc.tensor.matmul(ps0, lhsT, wo_t[ch][:, 0:512],
                                             start=(ch == 0), stop=(ch == NCH - 1))
                            nc.tensor.matmul(ps1, lhsT, wo_t[ch][:, 512:768],
                                             start=(ch == 0), stop=(ch == NCH - 1))
                            yield
                        ot = out_p.tile([128, E], F32, tag="outp")
                        nc.vector.tensor_tensor(out=ot[:, 0:512], in0=ps0,
                                                in1=bos[:, 0:512],
                                                op=mybir.AluOpType.add)
                        nc.vector.tensor_tensor(out=ot[:, 512:768], in0=ps1,
                                                in1=bos[:, 512:768],
                                                op=mybir.AluOpType.add)
                        eng = nc.sync if tcn % 2 == 0 else nc.scalar
                        eng.dma_start(
                            out=out[tcn * 128:(tcn + 1) * 128, :], in_=ot)

                def drain(gen):
                    for _ in gen:
                        pass

                # --- emission ---
                nc.gpsimd.dma_start(out=bqs, in_=bq_pm[:, :])
                nc.gpsimd.dma_start(out=bks, in_=bk_pm[:, :])

                q0 = proj_qk_steps(0, wq_r, bqs, qt_sb, "dve")
                k0 = proj_qk_steps(0, wk_r, bks, kt_sb, "dve")
                next(q0)  # emits the wq(0) DMA + first matmul

                m2 = mask2.rearrange("p s q -> p (s q)")
                nc.sync.dma_start(out=m2[:, 0:128], in_=maskc[:, :])
                nc.sync.dma_start(out=m2[:, 128:256], in_=maskc[:, :])
                nc.scalar.dma_start(out=bos, in_=bo_bc[:, :])

                drain(q0)
                drain(k0)

                wv_t = {}
                for ch in range(NCH):
                    w = wrow_p.tile([128, E], BF16, tag="wrow")
                    nc.sync.dma_start(
                        out=w, in_=wvT[ch * 128:(ch + 1) * 128, :])
                    wv_t[ch] = w
                for kc in range(NTC):
                    v65_r = v65_sb[:, kc, :].rearrange("p (h e) -> p h e", e=65)
                    nc.gpsimd.dma_start(out=v65_r[:, :, 64:65], in_=ones12[:, :])

                drain(proj_v_steps(range(0, 4)))
                _mix(attn_pair_steps(0, wins=(0,)), proj_v_steps(range(4, 8)), nf=3)
                _mix(attn_pair_steps(0, wins=(1,)),
                     _chain(proj_qk_steps(1, wq_r, bqs, qt_sb, "dve"),
                            proj_qk_steps(1, wk_r, bks, kt_sb, "dve")), nf=2)
                for m in range(1, NCH):
                    if m < NCH - 1:
                        filler = _chain(
                            proj_qk_steps(m + 1, wq_r, bqs, qt_sb, "dve"),
                            proj_qk_steps(m + 1, wk_r, bks, kt_sb, "dve"))
                        _mix(attn_pair_steps(m), filler, nf=2)
                    else:
                        # last pair: win0 alone, win1 overlapped with the
                        # first half of the output projection (reads only
                        # ctxT window-0 columns, complete for all pairs)
                        wo_t = {}
                        for ch in range(NCH):
                            w = wrow_p.tile([128, E], BF16, tag="wrow")
                            nc.sync.dma_start(
                                out=w, in_=woT[ch * 128:(ch + 1) * 128, :])
                            wo_t[ch] = w
                        drain(attn_pair_steps(m, wins=(0,)))
                        _mix(attn_pair_steps(m, wins=(1,)),
                             outproj_steps(range(0, 4)), nf=2)
                drain(outproj_steps(range(4, NTC)))

            for _rep in range(repeat):
                body()

    _split_excess_waits(nc)
    return nc


_NC_CACHE = None


def _make_in_maps(x, Wq, bq, Wk, bk, Wv, bv, Wo, bo):
    wq_r = _bf16(
        Wq.T.astype(np.float32).reshape(NCH, 128, NCH, 128).transpose(2, 1, 0, 3))
    wk_r = _bf16(
        Wk.T.astype(np.float32).reshape(NCH, 128, NCH, 128).transpose(2, 1, 0, 3))
    wvT = _bf16(Wv.T.astype(np.float32))
    woT = _bf16(Wo.T.astype(np.float32))
    bq_pm = np.ascontiguousarray(bq.reshape(NCH, 128).T.astype(np.float32))
    bk_pm = np.ascontiguousarray(bk.reshape(NCH, 128).T.astype(np.float32))
    bo_eff = (bo.astype(np.float64)
              + bv.astype(np.float64) @ Wo.T.astype(np.float64)).astype(np.float32)
    bo_bc = np.ascontiguousarray(np.tile(bo_eff[None, :], (128, 1)))
    maps = []
    for b in range(B):
        xTb = _bf16(x[b].T)
        maps.append({"xT": xTb, "wq_r": wq_r, "wk_r": wk_r, "wvT": wvT,
                     "woT": woT, "bq_pm": bq_pm, "bk_pm": bk_pm, "bo_bc": bo_bc})
    return maps


def kernel(x, Wq, bq, Wk, bk, Wv, bv, Wo, bo):
    global _NC_CACHE
    from concourse.bass_utils import run_bass_kernel_spmd
    if _NC_CACHE is None:
        _NC_CACHE = build_nc()
    in_maps = _make_in_maps(x, Wq, bq, Wk, bk, Wv, bv, Wo, bo)
    res = run_bass_kernel_spmd(_NC_CACHE, in_maps, core_ids=list(range(B)))
    return np.stack([res.results[i]["out"] for i in range(B)], axis=0)


# revision 32
# speedup vs baseline: 1.0688x; 1.0688x over previous
"""Causal multi-head attention (B=8, T=1024, E=768, H=12, D=64) on 8 trn2
NeuronCores, data-parallel over the batch (one batch element per core).

v3: all-bf16 matmuls + software-pipelined emission. The attention inner
loop is ACT(exp)-bound (~11us/pair vs ~7.7us PE), so the next pair's
Q/K projection matmuls are interleaved into the attention emission as PE
filler, and each kc's ctx matmul is emitted 2 steps after its scores so
the exp has drained by the time PE reaches it (the PE queue is in-order).
Softmax denominators ride as a ones column in V ("V65"); both heads'
reciprocal rows are batched into ONE DRAM round-trip per window (sc
issued from the sync queue, bc + normalize mults on Pool) so only a
single DMA-completion wait sits on the Pool queue per window.

Per-core pipeline:
  1. Q^T = Wq @ x^T + bq (bias+cast ACT), K^T likewise (DVE) -> bf16
     [768, 1024] (e_out on partitions).
  2. V = x @ Wv^T -> bf16 "V65" [1024, 12*65] with a ones column per head
     so the ctx matmul also produces the softmax denominator (bv folded
     into the output projection bias).
  3. Per head pair: S^T[k,q] row-packed K=64 matmuls, causal mask added on
     the diagonal block (single strided DVE op over both heads), exp on
     ACT (scale=1/8) -> bf16 P^T, ctx^T[65,q] accumulated with V65
     stationary. Row 64 = denominator; reciprocal (DVE) -> DRAM-bounce
     broadcast -> normalize mult (Pool, out bf16).
  4. out = ctx_norm @ Wo^T + bo_eff, bo_eff = bo + bv @ Wo^T.
"""
import sys
import numpy as np

sys.path.insert(0, "/opt/trn_rl_repo")

import concourse.bass as bass
import concourse.mybir as mybir
import concourse.tile as tile

F32 = mybir.dt.float32
BF16 = mybir.dt.bfloat16

B, T, E, H, D = 8, 1024, 768, 12, 64
NCH = E // 128          # 6 e-chunks
NTC = T // 128          # 8 t-chunks
NW = T // 512           # 2 q-windows
SCALE = 1.0 / np.sqrt(D)
NEG = -1.0e9


def _bf16(a):
    import ml_dtypes
    return np.ascontiguousarray(a.astype(ml_dtypes.bfloat16))


def _split_excess_waits(nc, max_waits: int = 1):
    """walrus on this stack accepts at most one embedded sync-wait per
    instruction; peel extras onto wait-only NoOps on the same engine."""
    for func in nc.m.functions:
        for bb in func.blocks:
            insts = bb.instructions
            i = 0
            while i < len(insts):
                inst = insts[i]
                si = getattr(inst, "sync_info", None)
                if si is None or len(si.on_wait) <= max_waits:
                    i += 1
                    continue
                waits = list(si.on_wait)
                keep, extra = waits[:max_waits], waits[max_waits:]
                nops = []
                while extra:
                    chunk, extra = extra[:max_waits], extra[max_waits:]
                    nop = mybir.InstNoOp(
                        name=f"{inst.name}_ws{len(nops)}", ins=[], outs=[])
                    nop.engine = inst.engine
                    nop.sync_info = mybir.SyncInfo(on_wait=chunk, on_update=[])
                    nc.register_instruction(nop, overwrite=True)
                    nops.append(nop)
                si.on_wait = keep
                for j, nop in enumerate(nops):
                    insts.insert(i + j, nop)
                i += len(nops) + 1


def _mix(main_gen, filler_gen, nf=2):
    """Pull 1 step from main, then up to nf from filler, repeat; drain both."""
    main_it, fill_it = iter(main_gen), iter(filler_gen)
    main_done = fill_done = False
    while not (main_done and fill_done):
        if not main_done:
            try:
                next(main_it)
            except StopIteration:
                main_done = True
        if not fill_done:
            for _ in range(nf):
                try:
                    next(fill_it)
                except StopIteration:
                    fill_done = True
                    break


def _chain(*gens):
    for g in gens:
        yield from g


def build_nc(repeat: int = 1):
    import ml_dtypes
    nc = bass.Bass()
    xT = nc.dram_tensor("xT", [E, T], BF16, kind="ExternalInput")
    wq_r = nc.dram_tensor("wq_r", [NCH, 128, NCH, 128], BF16, kind="ExternalInput")
    wk_r = nc.dram_tensor("wk_r", [NCH, 128, NCH, 128], BF16, kind="ExternalInput")
    wvT = nc.dram_tensor("wvT", [E, E], BF16, kind="ExternalInput")
    woT = nc.dram_tensor("woT", [E, E], BF16, kind="ExternalInput")
    bq_pm = nc.dram_tensor("bq_pm", [128, NCH], F32, kind="ExternalInput")
    bk_pm = nc.dram_tensor("bk_pm", [128, NCH], F32, kind="ExternalInput")
    bo_bc = nc.dram_tensor("bo_bc", [128, E], F32, kind="ExternalInput")
    out = nc.dram_tensor("out", [T, E], F32, kind="ExternalOutput")

    tril = np.where(np.arange(128)[None, :] >= np.arange(128)[:, None],
                    0.0, NEG).astype(np.float32)
    maskc = nc.inline_tensor(tril, name="maskc")
    ones12 = nc.inline_tensor(np.ones((128, H), ml_dtypes.bfloat16),
                              name="ones12")

    with tile.TileContext(nc) as tc:
        from contextlib import ExitStack
        with ExitStack() as ctx:
            consts = ctx.enter_context(tc.tile_pool(name="consts", bufs=1))
            persist = ctx.enter_context(tc.tile_pool(name="persist", bufs=1))
            wqk_p = ctx.enter_context(tc.tile_pool(name="wqk", bufs=3))
            wrow_p = ctx.enter_context(tc.tile_pool(name="wrow", bufs=6))
            pt_p = ctx.enter_context(tc.tile_pool(name="pt", bufs=5))
            bc_p = ctx.enter_context(tc.tile_pool(name="bc", bufs=4))
            cu_p = ctx.enter_context(tc.tile_pool(name="cu", bufs=6))
            rt_p = ctx.enter_context(tc.tile_pool(name="rt", bufs=4))
            out_p = ctx.enter_context(tc.tile_pool(name="outp", bufs=3))
            pp = ctx.enter_context(tc.tile_pool(name="pp", bufs=2, space="PSUM"))
            stp = ctx.enter_context(tc.tile_pool(name="stp", bufs=2, space="PSUM"))
            ctxp = ctx.enter_context(tc.tile_pool(name="ctxp", bufs=2, space="PSUM"))
            drp = ctx.enter_context(tc.tile_pool(name="drp", bufs=4, space="DRAM"))

            def body():
                mask2 = consts.tile([128, 2, 128], F32)
                bqs = consts.tile([128, NCH], F32)
                bks = consts.tile([128, NCH], F32)
                bos = consts.tile([128, E], F32)

                xt_sb = persist.tile([128, NCH, T], BF16)
                xq_engines = [nc.sync, nc.gpsimd, nc.scalar,
                              nc.sync, nc.gpsimd, nc.scalar]
                for ch in range(NCH):
                    xq_engines[ch].dma_start(
                        out=xt_sb[:, ch, :],
                        in_=xT[ch * 128:(ch + 1) * 128, :])

                qt_sb = persist.tile([128, NCH, T], BF16)
                kt_sb = persist.tile([128, NCH, T], BF16)
                v65_sb = persist.tile([128, NTC, H * 65], BF16)
                ctxT_sb = persist.tile([128, NCH, T], BF16)

                def proj_qk_steps(m, w_r, bias_sb, dst_sb, copy_eng):
                    w = wqk_p.tile([128, NCH, 128], BF16, tag="wqk")
                    nc.sync.dma_start(out=w, in_=w_r[m, :, :, :])
                    for win in range(NW):
                        ps = pp.tile([128, 512], F32, tag="pp")
                        for ch in range(NCH):
                            nc.tensor.matmul(
                                ps, w[:, ch, :],
                                xt_sb[:, ch, win * 512:(win + 1) * 512],
                                start=(ch == 0), stop=(ch == NCH - 1))
                            yield
                        if copy_eng == "act":
                            nc.scalar.activation(
                                dst_sb[:, m, win * 512:(win + 1) * 512], ps,
                                mybir.ActivationFunctionType.Identity,
                                bias=bias_sb[:, m:m + 1])
                        else:
                            nc.vector.tensor_scalar_add(
                                dst_sb[:, m, win * 512:(win + 1) * 512], ps,
                                bias_sb[:, m:m + 1])

                def proj_v_steps(kcs):
                    for kc in kcs:
                        ps0 = pp.tile([128, 512], F32, tag="pp")
                        ps1 = pp.tile([128, 256], F32, tag="pp")
                        for ch in range(NCH):
                            lhsT = xt_sb[:, ch, kc * 128:(kc + 1) * 128]
                            nc.tensor.matmul(ps0, lhsT, wv_t[ch][:, 0:512],
                                             start=(ch == 0), stop=(ch == NCH - 1))
                            nc.tensor.matmul(ps1, lhsT, wv_t[ch][:, 512:768],
                                             start=(ch == 0), stop=(ch == NCH - 1))
                            yield
                        v65_r = v65_sb[:, kc, :].rearrange("p (h e) -> p h e", e=65)
                        if kc % 2 == 0:
                            nc.scalar.copy(v65_r[:, 0:8, 0:64], ps0)
                            nc.scalar.copy(v65_r[:, 8:12, 0:64], ps1)
                        else:
                            nc.vector.tensor_copy(v65_r[:, 0:8, 0:64], ps0)
                            nc.vector.tensor_copy(v65_r[:, 8:12, 0:64], ps1)

                def attn_pair_steps(p, wins=tuple(range(NW)), delay=3):
                    for win in wins:
                        nk = 4 * (win + 1)
                        ctxA = ctxp.tile([65, 512], F32, tag="ctx")
                        ctxB = ctxp.tile([65, 512], F32, tag="ctx")
                        hA, hB = 2 * p, 2 * p + 1

                        def emit_ctx(kc, off, pt):
                            nc.tensor.matmul(
                                ctxA[:, off:512],
                                v65_sb[:, kc, hA * 65:hA * 65 + 65],
                                pt[:, off:512],
                                start=(kc == 0), stop=(kc == nk - 1))
                            nc.tensor.matmul(
                                ctxB[:, off:512],
                                v65_sb[:, kc, hB * 65:hB * 65 + 65],
                                pt[:, 512 + off:1024],
                                start=(kc == 0), stop=(kc == nk - 1))

                        pend = []
                        for kc in range(nk):
                            off = max(kc * 128 - win * 512, 0)
                            w0 = win * 512
                            st = stp.tile([128, 1024], F32, tag="st")
                            nc.tensor.matmul(
                                st[:, off:512],
                                kt_sb[0:64, p, kc * 128:(kc + 1) * 128],
                                qt_sb[0:64, p, w0 + off:w0 + 512],
                                start=True, stop=True, tile_position=(0, 0))
                            nc.tensor.matmul(
                                st[:, 512 + off:1024],
                                kt_sb[64:128, p, kc * 128:(kc + 1) * 128],
                                qt_sb[64:128, p, w0 + off:w0 + 512],
                                start=True, stop=True, tile_position=(64, 0))
                            if kc * 128 - win * 512 >= 0:
                                st3 = st.rearrange("p (s q) -> p s q", s=2)
                                nc.vector.tensor_tensor(
                                    out=st3[:, :, off:off + 128],
                                    in0=st3[:, :, off:off + 128],
                                    in1=mask2, op=mybir.AluOpType.add)
                            pt = pt_p.tile([128, 1024], BF16, tag="pt")
                            if off > 0:
                                st3 = st.rearrange("p (s q) -> p s q", s=2)
                                pt3 = pt.rearrange("p (s q) -> p s q", s=2)
                                nc.scalar.activation(
                                    pt3[:, :, off:512], st3[:, :, off:512],
                                    mybir.ActivationFunctionType.Exp, scale=SCALE)
                            else:
                                nc.scalar.activation(
                                    pt, st, mybir.ActivationFunctionType.Exp,
                                    scale=SCALE)
                            pend.append((kc, off, pt))
                            if len(pend) > delay:
                                emit_ctx(*pend.pop(0))
                            yield
                        while pend:
                            emit_ctx(*pend.pop(0))
                            yield
                        # one DRAM round-trip per window for BOTH heads:
                        # recips into one [1,1024] row, sc issued from the
                        # sync queue (no wait), bc + mults on Pool (a single
                        # wait-for-sc per window instead of two chains)
                        rt2 = rt_p.tile([1, 1024], F32, tag="rt")
                        # both recips first so the bounce DMA (which only
                        # needs rt2) isn't queued behind a cu copy on DVE
                        for idx, cps in ((0, ctxA), (1, ctxB)):
                            nc.vector.reciprocal(
                                rt2[:, idx * 512:(idx + 1) * 512],
                                cps[64:65, :])
                        sc = drp.tile([1, 1024], F32, tag="dr")
                        nc.sync.dma_start(out=sc, in_=rt2)
                        cus = []
                        for idx, cps in ((0, ctxA), (1, ctxB)):
                            cu = cu_p.tile([65, 512], F32, tag="cu")
                            nc.vector.tensor_copy(cu, cps)
                            cus.append(cu)
                        bc = bc_p.tile([64, 1024], F32, tag="bc")
                        sc_b = bass.AP(tensor=sc.tensor, offset=sc.offset,
                                       ap=[[0, 64]] + list(sc.ap)[1:])
                        nc.gpsimd.dma_start(out=bc, in_=sc_b)
                        for idx in (0, 1):
                            nc.gpsimd.tensor_tensor(
                                out=ctxT_sb[idx * 64:idx * 64 + 64, p,
                                            win * 512:(win + 1) * 512],
                                in0=cus[idx][0:64, :],
                                in1=bc[:, idx * 512:(idx + 1) * 512],
                                op=mybir.AluOpType.mult)
                        yield

                def outproj_steps(tcns):
                    for tcn in tcns:
                        ps0 = pp.tile([128, 512], F32, tag="pp")
                        ps1 = ctxp.tile([128, 256], F32, tag="ctx")
                        for ch in range(NCH):
                            lhsT = ctxT_sb[:, ch, tcn * 128:(tcn + 1) * 128]
                            nc.tensor.matmul(ps0, lhsT, wo_t[ch][:, 0:512],
                                             start=(ch == 0), stop=(ch == NCH - 1))
                            nc.tensor.matmul(ps1, lhsT, wo_t[ch][:, 512:768],
                                             start=(ch == 0), stop=(ch == NCH - 1))
                            yield
                        ot = out_p.tile([128, E], F32, tag="outp")
                        nc.vector.tensor_tensor(out=ot[:, 0:512], in0=ps0,
                                                in1=bos[:, 0:512],
                                                op=mybir.AluOpType.add)
                        nc.vector.tensor_tensor(out=ot[:, 512:768], in0=ps1,
                                                in1=bos[:, 512:768],
                                                op=mybir.AluOpType.add)
                        eng = nc.sync if tcn % 2 == 0 else nc.scalar
                        eng.dma_start(
                            out=out[tcn * 128:(tcn + 1) * 128, :], in_=ot)

                def drain(gen):
                    for _ in gen:
                        pass

                # --- emission ---
                nc.gpsimd.dma_start(out=bqs, in_=bq_pm[:, :])
                nc.gpsimd.dma_start(out=bks, in_=bk_pm[:, :])

                q0 = proj_qk_steps(0, wq_r, bqs, qt_sb, "act")
                k0 = proj_qk_steps(0, wk_r, bks, kt_sb, "dve")
                next(q0)  # emits the wq(0) DMA + first matmul

                m2 = mask2.rearrange("p s q -> p (s q)")
                nc.sync.dma_start(out=m2[:, 0:128], in_=maskc[:, :])
                nc.sync.dma_start(out=m2[:, 128:256], in_=maskc[:, :])
                nc.scalar.dma_start(out=bos, in_=bo_bc[:, :])

                drain(q0)
                drain(k0)

                wv_t = {}
                for ch in range(NCH):
                    w = wrow_p.tile([128, E], BF16, tag="wrow")
                    nc.sync.dma_start(
                        out=w, in_=wvT[ch * 128:(ch + 1) * 128, :])
                    wv_t[ch] = w
                for kc in range(NTC):
                    v65_r = v65_sb[:, kc, :].rearrange("p (h e) -> p h e", e=65)
                    nc.gpsimd.dma_start(out=v65_r[:, :, 64:65], in_=ones12[:, :])

                drain(proj_v_steps(range(0, 4)))
                _mix(attn_pair_steps(0, wins=(0,)), proj_v_steps(range(4, 8)), nf=3)
                _mix(attn_pair_steps(0, wins=(1,)),
                     _chain(proj_qk_steps(1, wq_r, bqs, qt_sb, "act"),
                            proj_qk_steps(1, wk_r, bks, kt_sb, "dve")), nf=2)
                for m in range(1, NCH):
                    if m < NCH - 1:
                        filler = _chain(
                            proj_qk_steps(m + 1, wq_r, bqs, qt_sb, "act"),
                            proj_qk_steps(m + 1, wk_r, bks, kt_sb, "dve"))
                        _mix(attn_pair_steps(m), filler, nf=2)
                    else:
                        # last pair: win0 alone, win1 overlapped with the
                        # first half of the output projection (reads only
                        # ctxT window-0 columns, complete for all pairs)
                        wo_t = {}
                        for ch in range(NCH):
                            w = wrow_p.tile([128, E], BF16, tag="wrow")
                            nc.sync.dma_start(
                                out=w, in_=woT[ch * 128:(ch + 1) * 128, :])
                            wo_t[ch] = w
                        drain(attn_pair_steps(m, wins=(0,)))
                        _mix(attn_pair_steps(m, wins=(1,)),
                             outproj_steps(range(0, 4)), nf=2)
                drain(outproj_steps(range(4, NTC)))

            for _rep in range(repeat):
                body()

    _split_excess_waits(nc)
    return nc


_NC_CACHE = None


def _make_in_maps(x, Wq, bq, Wk, bk, Wv, bv, Wo, bo):
    wq_r = _bf16(
        Wq.T.astype(np.float32).reshape(NCH, 128, NCH, 128).transpose(2, 1, 0, 3))
    wk_r = _bf16(
        Wk.T.astype(np.float32).reshape(NCH, 128, NCH, 128).transpose(2, 1, 0, 3))
    wvT = _bf16(Wv.T.astype(np.float32))
    woT = _bf16(Wo.T.astype(np.float32))
    bq_pm = np.ascontiguousarray(bq.reshape(NCH, 128).T.astype(np.float32))
    bk_pm = np.ascontiguousarray(bk.reshape(NCH, 128).T.astype(np.float32))
    bo_eff = (bo.astype(np.float64)
              + bv.astype(np.float64) @ Wo.T.astype(np.float64)).astype(np.float32)
    bo_bc = np.ascontiguousarray(np.tile(bo_eff[None, :], (128, 1)))
    maps = []
    for b in range(B):
        xTb = _bf16(x[b].T)
        maps.append({"xT": xTb, "wq_r": wq_r, "wk_r": wk_r, "wvT": wvT,
                     "woT": woT, "bq_pm": bq_pm, "bk_pm": bk_pm, "bo_bc": bo_bc})
    return maps


def kernel(x, Wq, bq, Wk, bk, Wv, bv, Wo, bo):
    global _NC_CACHE
    from concourse.bass_utils import run_bass_kernel_spmd
    if _NC_CACHE is None:
        _NC_CACHE = build_nc()
    in_maps = _make_in_maps(x, Wq, bq, Wk, bk, Wv, bv, Wo, bo)
    res = run_bass_kernel_spmd(_NC_CACHE, in_maps, core_ids=list(range(B)))
    return np.stack([res.results[i]["out"] for i in range(B)], axis=0)


# revision 33
# speedup vs baseline: 1.1106x; 1.0392x over previous
"""Causal multi-head attention (B=8, T=1024, E=768, H=12, D=64) on 8 trn2
NeuronCores, data-parallel over the batch (one batch element per core).

v3: all-bf16 matmuls + software-pipelined emission. The attention inner
loop is ACT(exp)-bound (~11us/pair vs ~7.7us PE), so the next pair's
Q/K projection matmuls are interleaved into the attention emission as PE
filler, and each kc's ctx matmul is emitted 2 steps after its scores so
the exp has drained by the time PE reaches it (the PE queue is in-order).
Softmax denominators ride as a ones column in V ("V65"); both heads'
reciprocal rows are batched into ONE DRAM round-trip per window (sc
issued from the sync queue, bc + normalize mults on Pool) so only a
single DMA-completion wait sits on the Pool queue per window.

Per-core pipeline:
  1. Q^T = Wq @ x^T + bq (bias+cast ACT), K^T likewise (DVE) -> bf16
     [768, 1024] (e_out on partitions).
  2. V = x @ Wv^T -> bf16 "V65" [1024, 12*65] with a ones column per head
     so the ctx matmul also produces the softmax denominator (bv folded
     into the output projection bias).
  3. Per head pair: S^T[k,q] row-packed K=64 matmuls, causal mask added on
     the diagonal block (single strided DVE op over both heads), exp on
     ACT (scale=1/8) -> bf16 P^T, ctx^T[65,q] accumulated with V65
     stationary. Row 64 = denominator; reciprocal (DVE) -> DRAM-bounce
     broadcast -> normalize mult (Pool, out bf16).
  4. out = ctx_norm @ Wo^T + bo_eff, bo_eff = bo + bv @ Wo^T.
"""
import sys
import numpy as np

sys.path.insert(0, "/opt/trn_rl_repo")

import concourse.bass as bass
import concourse.mybir as mybir
import concourse.tile as tile

F32 = mybir.dt.float32
BF16 = mybir.dt.bfloat16

B, T, E, H, D = 8, 1024, 768, 12, 64
NCH = E // 128          # 6 e-chunks
NTC = T // 128          # 8 t-chunks
NW = T // 512           # 2 q-windows
SCALE = 1.0 / np.sqrt(D)
NEG = -1.0e9


def _bf16(a):
    import ml_dtypes
    return np.ascontiguousarray(a.astype(ml_dtypes.bfloat16))


def _split_excess_waits(nc, max_waits: int = 1):
    """walrus on this stack accepts at most one embedded sync-wait per
    instruction; peel extras onto wait-only NoOps on the same engine."""
    for func in nc.m.functions:
        for bb in func.blocks:
            insts = bb.instructions
            i = 0
            while i < len(insts):
                inst = insts[i]
                si = getattr(inst, "sync_info", None)
                if si is None or len(si.on_wait) <= max_waits:
                    i += 1
                    continue
                waits = list(si.on_wait)
                keep, extra = waits[:max_waits], waits[max_waits:]
                nops = []
                while extra:
                    chunk, extra = extra[:max_waits], extra[max_waits:]
                    nop = mybir.InstNoOp(
                        name=f"{inst.name}_ws{len(nops)}", ins=[], outs=[])
                    nop.engine = inst.engine
                    nop.sync_info = mybir.SyncInfo(on_wait=chunk, on_update=[])
                    nc.register_instruction(nop, overwrite=True)
                    nops.append(nop)
                si.on_wait = keep
                for j, nop in enumerate(nops):
                    insts.insert(i + j, nop)
                i += len(nops) + 1


def _mix(main_gen, filler_gen, nf=2):
    """Pull 1 step from main, then up to nf from filler, repeat; drain both."""
    main_it, fill_it = iter(main_gen), iter(filler_gen)
    main_done = fill_done = False
    while not (main_done and fill_done):
        if not main_done:
            try:
                next(main_it)
            except StopIteration:
                main_done = True
        if not fill_done:
            for _ in range(nf):
                try:
                    next(fill_it)
                except StopIteration:
                    fill_done = True
                    break


def _chain(*gens):
    for g in gens:
        yield from g


def build_nc(repeat: int = 1):
    import ml_dtypes
    nc = bass.Bass()
    xT = nc.dram_tensor("xT", [E, T], BF16, kind="ExternalInput")
    wq_r = nc.dram_tensor("wq_r", [NCH, 128, NCH, 128], BF16, kind="ExternalInput")
    wk_r = nc.dram_tensor("wk_r", [NCH, 128, NCH, 128], BF16, kind="ExternalInput")
    wvT = nc.dram_tensor("wvT", [E, E], BF16, kind="ExternalInput")
    woT = nc.dram_tensor("woT", [E, E], BF16, kind="ExternalInput")
    bq_pm = nc.dram_tensor("bq_pm", [128, NCH], F32, kind="ExternalInput")
    bk_pm = nc.dram_tensor("bk_pm", [128, NCH], F32, kind="ExternalInput")
    bo_bc = nc.dram_tensor("bo_bc", [128, E], F32, kind="ExternalInput")
    out = nc.dram_tensor("out", [T, E], F32, kind="ExternalOutput")

    tril = np.where(np.arange(128)[None, :] >= np.arange(128)[:, None],
                    0.0, NEG).astype(np.float32)
    maskc = nc.inline_tensor(tril, name="maskc")
    ones12 = nc.inline_tensor(np.ones((128, H), ml_dtypes.bfloat16),
                              name="ones12")

    with tile.TileContext(nc) as tc:
        from contextlib import ExitStack
        with ExitStack() as ctx:
            consts = ctx.enter_context(tc.tile_pool(name="consts", bufs=1))
            persist = ctx.enter_context(tc.tile_pool(name="persist", bufs=1))
            wqk_p = ctx.enter_context(tc.tile_pool(name="wqk", bufs=3))
            wrow_p = ctx.enter_context(tc.tile_pool(name="wrow", bufs=6))
            pt_p = ctx.enter_context(tc.tile_pool(name="pt", bufs=5))
            bc_p = ctx.enter_context(tc.tile_pool(name="bc", bufs=4))
            cu_p = ctx.enter_context(tc.tile_pool(name="cu", bufs=6))
            rt_p = ctx.enter_context(tc.tile_pool(name="rt", bufs=4))
            out_p = ctx.enter_context(tc.tile_pool(name="outp", bufs=3))
            pp = ctx.enter_context(tc.tile_pool(name="pp", bufs=2, space="PSUM"))
            stp = ctx.enter_context(tc.tile_pool(name="stp", bufs=2, space="PSUM"))
            ctxp = ctx.enter_context(tc.tile_pool(name="ctxp", bufs=2, space="PSUM"))
            drp = ctx.enter_context(tc.tile_pool(name="drp", bufs=4, space="DRAM"))

            def body():
                mask2 = consts.tile([128, 2, 128], F32)
                bqs = consts.tile([128, NCH], F32)
                bks = consts.tile([128, NCH], F32)
                bos = consts.tile([128, E], F32)

                xt_sb = persist.tile([128, NCH, T], BF16)
                xq_engines = [nc.sync, nc.gpsimd, nc.scalar,
                              nc.sync, nc.gpsimd, nc.scalar]
                for ch in range(NCH):
                    xq_engines[ch].dma_start(
                        out=xt_sb[:, ch, :],
                        in_=xT[ch * 128:(ch + 1) * 128, :])

                qt_sb = persist.tile([128, NCH, T], BF16)
                kt_sb = persist.tile([128, NCH, T], BF16)
                v65_sb = persist.tile([128, NTC, H * 65], BF16)
                ctxT_sb = persist.tile([128, NCH, T], BF16)

                def proj_qk_steps(m, w_r, bias_sb, dst_sb, copy_eng):
                    w = wqk_p.tile([128, NCH, 128], BF16, tag="wqk")
                    nc.sync.dma_start(out=w, in_=w_r[m, :, :, :])
                    for win in range(NW):
                        ps = pp.tile([128, 512], F32, tag="pp")
                        for ch in range(NCH):
                            nc.tensor.matmul(
                                ps, w[:, ch, :],
                                xt_sb[:, ch, win * 512:(win + 1) * 512],
                                start=(ch == 0), stop=(ch == NCH - 1))
                            yield
                        if copy_eng == "act":
                            nc.scalar.activation(
                                dst_sb[:, m, win * 512:(win + 1) * 512], ps,
                                mybir.ActivationFunctionType.Identity,
                                bias=bias_sb[:, m:m + 1])
                        else:
                            nc.vector.tensor_scalar_add(
                                dst_sb[:, m, win * 512:(win + 1) * 512], ps,
                                bias_sb[:, m:m + 1])

                def proj_v_steps(kcs):
                    for kc in kcs:
                        ps0 = pp.tile([128, 512], F32, tag="pp")
                        ps1 = pp.tile([128, 256], F32, tag="pp")
                        for ch in range(NCH):
                            lhsT = xt_sb[:, ch, kc * 128:(kc + 1) * 128]
                            nc.tensor.matmul(ps0, lhsT, wv_t[ch][:, 0:512],
                                             start=(ch == 0), stop=(ch == NCH - 1))
                            nc.tensor.matmul(ps1, lhsT, wv_t[ch][:, 512:768],
                                             start=(ch == 0), stop=(ch == NCH - 1))
                            yield
                        v65_r = v65_sb[:, kc, :].rearrange("p (h e) -> p h e", e=65)
                        if kc % 2 == 0:
                            nc.scalar.copy(v65_r[:, 0:8, 0:64], ps0)
                            nc.scalar.copy(v65_r[:, 8:12, 0:64], ps1)
                        else:
                            nc.vector.tensor_copy(v65_r[:, 0:8, 0:64], ps0)
                            nc.vector.tensor_copy(v65_r[:, 8:12, 0:64], ps1)

                def attn_pair_steps(p, wins=tuple(range(NW)), delay=3):
                    for win in wins:
                        nk = 4 * (win + 1)
                        ctxA = ctxp.tile([65, 512], F32, tag="ctx")
                        ctxB = ctxp.tile([65, 512], F32, tag="ctx")
                        hA, hB = 2 * p, 2 * p + 1

                        def emit_ctx(kc, off, pt):
                            nc.tensor.matmul(
                                ctxA[:, off:512],
                                v65_sb[:, kc, hA * 65:hA * 65 + 65],
                                pt[:, off:512],
                                start=(kc == 0), stop=(kc == nk - 1))
                            nc.tensor.matmul(
                                ctxB[:, off:512],
                                v65_sb[:, kc, hB * 65:hB * 65 + 65],
                                pt[:, 512 + off:1024],
                                start=(kc == 0), stop=(kc == nk - 1))

                        pend = []
                        for kc in range(nk):
                            off = max(kc * 128 - win * 512, 0)
                            w0 = win * 512
                            st = stp.tile([128, 1024], F32, tag="st")
                            nc.tensor.matmul(
                                st[:, off:512],
                                kt_sb[0:64, p, kc * 128:(kc + 1) * 128],
                                qt_sb[0:64, p, w0 + off:w0 + 512],
                                start=True, stop=True, tile_position=(0, 0))
                            nc.tensor.matmul(
                                st[:, 512 + off:1024],
                                kt_sb[64:128, p, kc * 128:(kc + 1) * 128],
                                qt_sb[64:128, p, w0 + off:w0 + 512],
                                start=True, stop=True, tile_position=(64, 0))
                            if kc * 128 - win * 512 >= 0:
                                st3 = st.rearrange("p (s q) -> p s q", s=2)
                                nc.vector.tensor_tensor(
                                    out=st3[:, :, off:off + 128],
                                    in0=st3[:, :, off:off + 128],
                                    in1=mask2, op=mybir.AluOpType.add)
                            pt = pt_p.tile([128, 1024], BF16, tag="pt")
                            if off > 0:
                                st3 = st.rearrange("p (s q) -> p s q", s=2)
                                pt3 = pt.rearrange("p (s q) -> p s q", s=2)
                                nc.scalar.activation(
                                    pt3[:, :, off:512], st3[:, :, off:512],
                                    mybir.ActivationFunctionType.Exp, scale=SCALE)
                            else:
                                nc.scalar.activation(
                                    pt, st, mybir.ActivationFunctionType.Exp,
                                    scale=SCALE)
                            pend.append((kc, off, pt))
                            if len(pend) > delay:
                                emit_ctx(*pend.pop(0))
                            yield
                        while pend:
                            emit_ctx(*pend.pop(0))
                            yield
                        # one DRAM round-trip per window for BOTH heads:
                        # recips into one [1,1024] row, sc issued from the
                        # sync queue (no wait), bc + mults on Pool (a single
                        # wait-for-sc per window instead of two chains)
                        rt2 = rt_p.tile([1, 1024], F32, tag="rt")
                        cus = []
                        for idx, cps in ((0, ctxA), (1, ctxB)):
                            nc.vector.reciprocal(
                                rt2[:, idx * 512:(idx + 1) * 512],
                                cps[64:65, :])
                            cu = cu_p.tile([65, 512], F32, tag="cu")
                            nc.vector.tensor_copy(cu, cps)
                            cus.append(cu)
                        sc = drp.tile([1, 1024], F32, tag="dr")
                        nc.sync.dma_start(out=sc, in_=rt2)
                        bc = bc_p.tile([64, 1024], F32, tag="bc")
                        sc_b = bass.AP(tensor=sc.tensor, offset=sc.offset,
                                       ap=[[0, 64]] + list(sc.ap)[1:])
                        nc.gpsimd.dma_start(out=bc, in_=sc_b)
                        for idx in (0, 1):
                            nc.gpsimd.tensor_tensor(
                                out=ctxT_sb[idx * 64:idx * 64 + 64, p,
                                            win * 512:(win + 1) * 512],
                                in0=cus[idx][0:64, :],
                                in1=bc[:, idx * 512:(idx + 1) * 512],
                                op=mybir.AluOpType.mult)
                        yield

                def outproj_steps(tcns):
                    for tcn in tcns:
                        ps0 = pp.tile([128, 512], F32, tag="pp")
                        ps1 = ctxp.tile([128, 256], F32, tag="ctx")
                        for ch in range(NCH):
                            lhsT = ctxT_sb[:, ch, tcn * 128:(tcn + 1) * 128]
                            nc.tensor.matmul(ps0, lhsT, wo_t[ch][:, 0:512],
                                             start=(ch == 0), stop=(ch == NCH - 1))
                            nc.tensor.matmul(ps1, lhsT, wo_t[ch][:, 512:768],
                                             start=(ch == 0), stop=(ch == NCH - 1))
                            yield
                        ot = out_p.tile([128, E], F32, tag="outp")
                        nc.vector.tensor_tensor(out=ot[:, 0:512], in0=ps0,
                                                in1=bos[:, 0:512],
                                                op=mybir.AluOpType.add)
                        nc.vector.tensor_tensor(out=ot[:, 512:768], in0=ps1,
                                                in1=bos[:, 512:768],
                                                op=mybir.AluOpType.add)
                        eng = nc.sync if tcn % 2 == 0 else nc.scalar
                        eng.dma_start(
                            out=out[tcn * 128:(tcn + 1) * 128, :], in_=ot)

                def drain(gen):
                    for _ in gen:
                        pass

                # --- emission ---
                nc.gpsimd.dma_start(out=bqs, in_=bq_pm[:, :])
                nc.gpsimd.dma_start(out=bks, in_=bk_pm[:, :])

                q0 = proj_qk_steps(0, wq_r, bqs, qt_sb, "act")
                k0 = proj_qk_steps(0, wk_r, bks, kt_sb, "dve")
                next(q0)  # emits the wq(0) DMA + first matmul

                m2 = mask2.rearrange("p s q -> p (s q)")
                nc.sync.dma_start(out=m2[:, 0:128], in_=maskc[:, :])
                nc.sync.dma_start(out=m2[:, 128:256], in_=maskc[:, :])
                nc.scalar.dma_start(out=bos, in_=bo_bc[:, :])

                drain(q0)
                drain(k0)

                wv_t = {}
                for ch in range(NCH):
                    w = wrow_p.tile([128, E], BF16, tag="wrow")
                    nc.sync.dma_start(
                        out=w, in_=wvT[ch * 128:(ch + 1) * 128, :])
                    wv_t[ch] = w
                for kc in range(NTC):
                    v65_r = v65_sb[:, kc, :].rearrange("p (h e) -> p h e", e=65)
                    nc.gpsimd.dma_start(out=v65_r[:, :, 64:65], in_=ones12[:, :])

                drain(proj_v_steps(range(0, 4)))
                _mix(attn_pair_steps(0, wins=(0,)), proj_v_steps(range(4, 8)), nf=3)
                _mix(attn_pair_steps(0, wins=(1,)),
                     _chain(proj_qk_steps(1, wq_r, bqs, qt_sb, "act"),
                            proj_qk_steps(1, wk_r, bks, kt_sb, "dve")), nf=2)
                for m in range(1, NCH):
                    if m < NCH - 1:
                        filler = _chain(
                            proj_qk_steps(m + 1, wq_r, bqs, qt_sb, "act"),
                            proj_qk_steps(m + 1, wk_r, bks, kt_sb, "dve"))
                        _mix(attn_pair_steps(m), filler, nf=2)
                    else:
                        # last pair: win0 alone, win1 overlapped with the
                        # first half of the output projection (reads only
                        # ctxT window-0 columns, complete for all pairs)
                        wo_t = {}
                        for ch in range(NCH):
                            w = wrow_p.tile([128, E], BF16, tag="wrow")
                            nc.sync.dma_start(
                                out=w, in_=woT[ch * 128:(ch + 1) * 128, :])
                            wo_t[ch] = w
                        drain(attn_pair_steps(m, wins=(0,)))
                        _mix(attn_pair_steps(m, wins=(1,)),
                             outproj_steps(range(0, 4)), nf=2)
                drain(outproj_steps(range(4, NTC)))

            for _rep in range(repeat):
                body()

    _split_excess_waits(nc)
    return nc


_NC_CACHE = None


def _make_in_maps(x, Wq, bq, Wk, bk, Wv, bv, Wo, bo):
    wq_r = _bf16(
        Wq.T.astype(np.float32).reshape(NCH, 128, NCH, 128).transpose(2, 1, 0, 3))
    wk_r = _bf16(
        Wk.T.astype(np.float32).reshape(NCH, 128, NCH, 128).transpose(2, 1, 0, 3))
    wvT = _bf16(Wv.T.astype(np.float32))
    woT = _bf16(Wo.T.astype(np.float32))
    bq_pm = np.ascontiguousarray(bq.reshape(NCH, 128).T.astype(np.float32))
    bk_pm = np.ascontiguousarray(bk.reshape(NCH, 128).T.astype(np.float32))
    bo_eff = (bo.astype(np.float64)
              + bv.astype(np.float64) @ Wo.T.astype(np.float64)).astype(np.float32)
    bo_bc = np.ascontiguousarray(np.tile(bo_eff[None, :], (128, 1)))
    maps = []
    for b in range(B):
        xTb = _bf16(x[b].T)
        maps.append({"xT": xTb, "wq_r": wq_r, "wk_r": wk_r, "wvT": wvT,
                     "woT": woT, "bq_pm": bq_pm, "bk_pm": bk_pm, "bo_bc": bo_bc})
    return maps


def kernel(x, Wq, bq, Wk, bk, Wv, bv, Wo, bo):
    global _NC_CACHE
    from concourse.bass_utils import run_bass_kernel_spmd
    if _NC_CACHE is None:
        _NC_CACHE = build_nc()
    in_maps = _make_in_maps(x, Wq, bq, Wk, bk, Wv, bv, Wo, bo)
    res = run_bass_kernel_spmd(_NC_CACHE, in_maps, core_ids=list(range(B)))
    return np.stack([res.results[i]["out"] for i in range(B)], axis=0)
